# revision 8
# baseline (speedup 1.0000x reference)
"""Multi-head attention (B=4, N=2048, DIM=512, H=8) on 8 TRN2 NeuronCores.

Sharding: core c handles (batch = c//2, query-half = c%2) -> 1024 queries of
one batch, all heads. Zero collectives: K/V are recomputed per core pair
(keys are permuted so each core's queries come first; softmax is
permutation-invariant over keys).

Device layout ("transposed flash"):
  - everything dim-major: X^T, Q^T, K^T in SBUF with the contraction dim on
    partitions; V in natural [n, d] layout with a fused ones-column so the
    PV matmul also produces the softmax denominators (row 64 of the PSUM
    accumulator).
  - scores S^T = K_h^T-stationary @ Q_h^T-moving -> [nk_tile, nq] PSUM.
    The two heads of a pair live on disjoint PE row groups (d on partitions
    0:64 vs 64:128); their score matmuls are emitted ADJACENTLY
    (A-c0, B-c0, A-c1, B-c1) so the PE runs them concurrently as 64x128
    row tiles (2x throughput -- verified on HW: 111 ns vs 217 ns per MM).
  - exp on ScalarE (scale fused) -> P^T bf16 in SBUF. A tunable subset of
    head-B tiles runs instead on VectorE via a custom 8-slice DVE op
    computing exp(x) ~= ((1 + u + u^2/2)^8, u = x/8 (rel err < 2e-3 over
    the logit range; softmax denominator cancels most of it), offloading
    the ScalarE bottleneck.
  - O^T_aug += V_aug^T-stationary @ P^T-moving accumulated over nk tiles.
  - normalization: reciprocal of the sums row, broadcast across partitions
    via a DRAM round-trip DMA (step-0 access pattern), multiply on VectorE.
  - output projection Wout-stationary gives Y^T [512, 1024]; host transposes.
"""

import os

import numpy as np
import ml_dtypes

B, N, DIM = 4, 2048, 512
H, D = 8, 64
NQ = 1024            # queries per core
NCORES = 8
SCALE = DIM ** -0.5  # reference scales by full dim, not head dim

BF16 = ml_dtypes.bfloat16

_CACHE = {}

LAST_EXEC_TIME_NS = None

# head-B's exp runs on the DVE for ALL k-tiles: symmetric engine use makes
# both ss PSUM slots free at the same time, so the next tile's A/B score
# matmuls become ready together and the PE pairs them (64x128 row tiles)
EXP8_C1 = 0.51        # tuned quadratic coefficient


def _register_exp8():
    """Register the EXP8_APPROX_ANT custom DVE op (idempotent).

    out = ((C1*u + 1) * u + 1)^8 with u = Src0 * C0; C0 folds the softmax
    scale / 8.  Exactly 8 ALU slices -- fits the v3 (TRN2) DVE pipeline.
    """
    import concourse.dve_ops as dve_ops
    from concourse.dve_spec import C0, C1, C2, Spec, lower, sq
    from concourse.dve_ops import DveOp
    from concourse.dve_uop import DveOpSpec

    if "EXP8_APPROX_ANT" in dve_ops._SUB_OPCODE_FOR_NAME:
        return next(op for op in dve_ops.OPS if op.name == "EXP8_APPROX_ANT")

    from concourse.dve_spec import Src0

    u = Src0 * C0
    body = sq(sq(sq((u * C1 + C2) * u + C2)))

    def _ref(in0, in1, s0, s1, imm2):
        uu = in0.astype(np.float32) * s0
        p = (uu * s1 + imm2) * uu + imm2
        return ((p * p) ** 2) ** 2

    spec = Spec(body=body, reference=_ref)

    row = max(dve_ops._SUB_OPCODE_FOR_NAME.values()) + 1
    assert row < 0x20

    # pin the sha by lowering once per version
    shas = {}
    for ver in ("v3", "v4"):
        try:
            uops = lower(spec, ver=ver)
            shas[ver] = DveOpSpec(
                name="EXP8_APPROX_ANT", opcode=row, uops=uops, rd1_en=False
            ).sha(ver)
        except Exception:
            pass

    op = DveOp("EXP8_APPROX_ANT", spec, subdim=False, uops_sha=shas)
    dve_ops.OPS.append(op)
    dve_ops.CUSTOM_DVE_SPECS[op.name] = spec
    dve_ops._SUB_OPCODE_FOR_NAME[op.name] = row
    return op


def _build():
    import concourse.bass as bass
    import concourse.mybir as mybir
    import concourse.tile as tile
    from concourse import bacc

    f32 = mybir.dt.float32
    bf16 = mybir.dt.bfloat16
    Exp = mybir.ActivationFunctionType.Exp
    ts = bass.ts

    exp8 = _register_exp8()
    exp8_s0 = SCALE / 8.0
    use_dve_exp = not bool(int(os.environ.get("ATTN_NO_DVE_EXP", "0")))

    nc = bacc.Bacc("TRN2", target_bir_lowering=False, debug=False,
                   num_devices=NCORES)

    xt = nc.dram_tensor("xt", [DIM, N], bf16, kind="ExternalInput")
    wqkv = nc.dram_tensor("wqkv", [DIM, 3 * DIM], bf16, kind="ExternalInput")
    wout = nc.dram_tensor("wout", [DIM, DIM], bf16, kind="ExternalInput")
    bout = nc.dram_tensor("bout", [128, 4], f32, kind="ExternalInput")
    yt = nc.dram_tensor("out", [DIM, NQ], f32, kind="ExternalOutput")

    with tile.TileContext(nc) as tc:
        with (
            tc.tile_pool(name="persist", bufs=1) as persist,
            tc.tile_pool(name="ptiles", bufs=8) as ptiles,
            tc.tile_pool(name="norm", bufs=2) as norm,
            tc.tile_pool(name="ysb", bufs=2) as ysb,
            tc.tile_pool(name="psum_s", bufs=1, space="PSUM") as psum_s,
            tc.tile_pool(name="psum_o", bufs=4, space="PSUM") as psum_o,
            tc.tile_pool(name="dram", bufs=2, space="DRAM") as dram,
        ):
            # ---- load inputs (fine-grained DMAs spread across queues) ----
            # Issue order = queue assignment order: land the chunks the first
            # projection (K tile 0, then Q, m=0) needs before everything else.
            xt_sb = persist.tile([128, 4, N], bf16)
            w_sb = persist.tile([128, 4, 3 * DIM], bf16)
            # critical path first, on distinct queues: the K m=0 / Q m=0
            # weight slivers and x^T (full rows; descriptor count is per
            # partition, so chunking columns does not speed a queue up)
            def ld(i, dst, srcs):
                eng = nc.sync if i % 2 == 0 else nc.scalar
                eng.dma_start(dst, srcs)
            for kt in range(4):
                ld(kt, w_sb[:, kt, 512:640], wqkv[ts(kt, 128), 512:640])
            for kt in range(4):
                ld(kt, xt_sb[:, kt, 0:1024], xt[ts(kt, 128), 0:1024])
            for kt in range(4):
                ld(kt + 1, w_sb[:, kt, 0:128], wqkv[ts(kt, 128), 0:128])
            for kt in range(4):
                ld(kt + 1, xt_sb[:, kt, 1024:2048], xt[ts(kt, 128), 1024:2048])
            for kt in range(4):
                ld(kt, w_sb[:, kt, ts(2, 512)], wqkv[ts(kt, 128), ts(2, 512)])
            for kt in range(4):
                ld(kt + 1, w_sb[:, kt, 640:1024], wqkv[ts(kt, 128), 640:1024])
            for kt in range(4):
                ld(kt, w_sb[:, kt, 128:512], wqkv[ts(kt, 128), 128:512])
            wout_sb = persist.tile([128, 4, DIM], bf16)
            for kt in range(4):
                ld(kt + 1, wout_sb[:, kt, :], wout[ts(kt, 128), :])
            bout_sb = persist.tile([128, 4], f32)
            nc.sync.dma_start(bout_sb[:], bout[:, :])

            qt_sb = persist.tile([128, 4, NQ], bf16)
            kt_sb = persist.tile([128, 4, N], bf16)
            # partition-swapped copies (head A on rows 64:128, B on rows
            # 0:64): chunk c0 of a head's score matmul runs on one PE row
            # half and chunk c1 on the other, so the two chunks pair on the
            # PE regardless of scheduler phase.
            qt_sw = persist.tile([128, 4, NQ], bf16)
            kt_sw = persist.tile([128, 4, N], bf16)

            def swap_kq(m):
                # SBUF->SBUF partition-shift DMAs, chunked for early tiles
                for c0 in range(0, N, 512):
                    nc.sync.dma_start(kt_sw[64:128, m, c0:c0 + 512],
                                      kt_sb[0:64, m, c0:c0 + 512])
                    nc.sync.dma_start(kt_sw[0:64, m, c0:c0 + 512],
                                      kt_sb[64:128, m, c0:c0 + 512])
                for c0 in range(0, NQ, 512):
                    nc.sync.dma_start(qt_sw[64:128, m, c0:c0 + 512],
                                      qt_sb[0:64, m, c0:c0 + 512])
                    nc.sync.dma_start(qt_sw[0:64, m, c0:c0 + 512],
                                      qt_sb[64:128, m, c0:c0 + 512])

            def q_proj(m, split_copy=False):
                # Q^T tile m: stationary = Wq tile, moving = X^T.
                # Two single-bank chunk tiles so the po ring stays 1-bank
                # granular (pv0/pv1/pden share the same 4-slot ring).
                ps0 = psum_o.tile([128, 512], mybir.dt.float32, tag="po",
                                  name=f"psq{m}c0")
                ps1 = psum_o.tile([128, 512], mybir.dt.float32, tag="po",
                                  name=f"psq{m}c1")
                pss = (ps0, ps1)
                for kt in range(4):
                    for c in range(2):
                        nc.tensor.matmul(
                            pss[c][:, :],
                            lhsT=w_sb[:, kt, ts(m, 128)],
                            rhs=xt_sb[:, kt, ts(c, 512)],
                            start=(kt == 0), stop=(kt == 3),
                        )
                if split_copy:
                    # first chunk unblocks the first score matmuls sooner
                    nc.vector.tensor_copy(qt_sb[:, m, 0:512], ps0[:, :])
                    nc.vector.tensor_copy(qt_sb[:, m, 512:NQ], ps1[:, :])
                else:
                    nc.scalar.copy(qt_sb[:, m, 0:512], ps0[:, :])
                    nc.scalar.copy(qt_sb[:, m, 512:NQ], ps1[:, :])

            def k_proj(m, cc, split_copy=False):
                ps0 = psum_o.tile([128, 512], mybir.dt.float32, tag="po",
                                  name=f"psk{m}_{cc}c0")
                ps1 = psum_o.tile([128, 512], mybir.dt.float32, tag="po",
                                  name=f"psk{m}_{cc}c1")
                pss = (ps0, ps1)
                for kt in range(4):
                    for c in range(2):
                        nc.tensor.matmul(
                            pss[c][:, :],
                            lhsT=w_sb[:, kt, 512 + m * 128:512 + (m + 1) * 128],
                            rhs=xt_sb[:, kt, cc * 1024 + c * 512:cc * 1024 + (c + 1) * 512],
                            start=(kt == 0), stop=(kt == 3),
                        )
                base = cc * 1024
                if split_copy:
                    nc.scalar.copy(kt_sb[:, m, base:base + 256],
                                   ps0[:, 0:256])
                    nc.scalar.copy(kt_sb[:, m, base + 256:base + 512],
                                   ps0[:, 256:512])
                    nc.scalar.copy(kt_sb[:, m, base + 512:base + 1024],
                                   ps1[:, :])
                else:
                    nc.scalar.copy(kt_sb[:, m, base:base + 512], ps0[:, :])
                    nc.scalar.copy(kt_sb[:, m, base + 512:base + 1024],
                                   ps1[:, :])

            k_proj(0, 0, split_copy=True)
            q_proj(0, split_copy=True)
            k_proj(0, 1)
            swap_kq(0)
            q_proj(1)
            k_proj(1, 0)
            k_proj(1, 1)
            swap_kq(1)

            # V natural [2048, 512] -> v_sb [128, nk_tile, head, 64]; the
            # softmax denominators come from separate ones-matmuls (so the
            # per-head PV stationary stays 64-wide and two heads pair on
            # disjoint PE column groups).
            v_sb = persist.tile([128, 16, H, D], bf16)
            ones_sb = persist.tile([128, 1], bf16)
            nc.vector.memset(ones_sb[:, :], 1.0)

            def v_proj():
                for t in range(16):
                    ps = psum_o.tile([128, 512], mybir.dt.float32, tag="po",
                                     name=f"psv{t}")
                    for kt in range(4):
                        nc.tensor.matmul(
                            ps[:, :],
                            lhsT=xt_sb[:, kt, ts(t, 128)],
                            rhs=w_sb[:, kt, 1024:1536],
                            start=(kt == 0), stop=(kt == 3),
                        )
                    if t % 2 == 0:
                        nc.scalar.copy(
                            v_sb[:, t, :, :],
                            ps[:, :].rearrange("p (h d) -> p h d", h=H),
                        )
                    else:
                        nc.vector.tensor_copy(
                            v_sb[:, t, :, :],
                            ps[:, :].rearrange("p (h d) -> p h d", h=H),
                        )

            # ---- attention, one head PAIR at a time ----
            # Head A lives on partitions 0:64, head B on 64:128 of K^T/Q^T
            # tile hp.  Both heads' scores for one k-tile land in ONE
            # [128, 2048] PSUM tile (A cols 0:1024, B cols 1024:2048): the
            # 4 score matmuls become ready atomically, so the scheduler
            # issues them back-to-back and the PE row-tiles A/B pairs
            # concurrently.  exp(A) on ScalarE reads banks 0-1 while
            # exp8(B) on VectorE reads banks 2-3 in parallel.
            ot_sb = persist.tile([128, 4, NQ], bf16)

            for hp in range(4):
                hA, hB = 2 * hp, 2 * hp + 1
                # po accumulators allocated lazily (for pair 0 they must come
                # AFTER the V-projection's psum_o allocations)
                pv0 = pv1 = pden = None
                # Software-pipelined: scores/exp for tile t are emitted one
                # iteration AHEAD of the PV matmuls for tile t-1, so freshly
                # unblocked score matmuls sit at the head of the PE FIFO
                # instead of behind the PV work (keeps ScalarE saturated).
                prev = None
                for t in range(17):
                    if t < 16:
                        # separate single-buffered tiles per head: the two
                        # exps run on different engines (ScalarE / VectorE)
                        # with no shared-tile reader chaining, so both slots
                        # free together and the next tile's A/B matmuls are
                        # ready simultaneously -> PE pairs them.
                        ssA = psum_s.tile([128, NQ], mybir.dt.float32,
                                          tag="ssA", name="ssA")
                        ssB = psum_s.tile([128, NQ], mybir.dt.float32,
                                          tag="ssB", name="ssB")
                        # sacrificial N=1 matmuls: they carry the PSUM-slot
                        # release waits (and the 128->64 tile-mode switch), so
                        # the real score matmuls below issue wait-free and the
                        # PE runs the A/B row-tile pairs concurrently
                        nc.tensor.matmul(
                            ssA[:, 0:1],
                            lhsT=kt_sb[0:64, hp, ts(t, 128)],
                            rhs=qt_sb[0:64, hp, 0:1],
                            start=True, stop=True,
                        )
                        nc.tensor.matmul(
                            ssB[:, 0:1],
                            lhsT=kt_sb[64:128, hp, ts(t, 128)],
                            rhs=qt_sb[64:128, hp, 0:1],
                            start=True, stop=True,
                        )
                        # c0 on one PE row half, c1 on the other (swapped
                        # copies) -> the two chunks run concurrently
                        nc.tensor.matmul(
                            ssA[:, 0:512],
                            lhsT=kt_sb[0:64, hp, ts(t, 128)],
                            rhs=qt_sb[0:64, hp, 0:512],
                            start=True, stop=True,
                        )
                        nc.tensor.matmul(
                            ssA[:, 512:1024],
                            lhsT=kt_sw[64:128, hp, ts(t, 128)],
                            rhs=qt_sw[64:128, hp, 512:1024],
                            start=True, stop=True,
                        )
                        nc.tensor.matmul(
                            ssB[:, 0:512],
                            lhsT=kt_sb[64:128, hp, ts(t, 128)],
                            rhs=qt_sb[64:128, hp, 0:512],
                            start=True, stop=True,
                        )
                        nc.tensor.matmul(
                            ssB[:, 512:1024],
                            lhsT=kt_sw[0:64, hp, ts(t, 128)],
                            rhs=qt_sw[0:64, hp, 512:1024],
                            start=True, stop=True,
                        )
                        ptA = ptiles.tile([128, NQ], bf16, tag="pt")
                        ptB = ptiles.tile([128, NQ], bf16, tag="pt")
                        if t == 0 and hp == 0:
                            # split the very first exp so ScalarE ramps sooner
                            nc.scalar.activation(ptA[:, 0:512], ssA[:, 0:512],
                                                 Exp, scale=SCALE)
                            nc.scalar.activation(ptA[:, 512:NQ], ssA[:, 512:NQ],
                                                 Exp, scale=SCALE)
                        else:
                            nc.scalar.activation(ptA[:, :], ssA[:, :], Exp,
                                                 scale=SCALE)
                        if use_dve_exp:
                            nc.vector._custom_dve(
                                exp8, out=ptB[:, :], in0=ssB[:, :],
                                s0=exp8_s0, s1=EXP8_C1, imm2=1.0,
                            )
                        else:
                            nc.scalar.activation(ptB[:, :], ssB[:, :], Exp,
                                                 scale=SCALE)
                    # V-projection rides here for pair 0: the first score
                    # group + exp are already emitted, so ScalarE ramps up
                    # while the PE grinds through the V matmuls.
                    if hp == 0 and t == 0:
                        v_proj()
                    if t >= 1:
                        if pv0 is None:
                            # pv0 = c0 bank (A rows 0:64, B rows 64:128),
                            # pv1 = c1 bank, pden = denominator bank with
                            # rows {0: A-c0, 32: B-c0, 64: A-c1, 96: B-c1}
                            pv0 = psum_o.tile([128, 512], mybir.dt.float32,
                                              tag="po", name="pv0")
                            pv1 = psum_o.tile([128, 512], mybir.dt.float32,
                                              tag="po", name="pv1")
                            pden = psum_o.tile([128, 512], mybir.dt.float32,
                                               tag="po", name="pden")
                        pA, pB = prev
                        st, sp_ = (t == 1), (t == 16)
                        # column-group-balanced schedule: each 32-col PE
                        # group sees exactly 3x512 moving columns per k-tile
                        # (two 64-row V matmuls + one 1-row ones matmul),
                        # emitted in col-disjoint pairs so they overlap.
                        nc.tensor.matmul(
                            pv0[0:64, :], lhsT=v_sb[:, t - 1, hA, :],
                            rhs=pA[:, 0:512], start=st, stop=sp_,
                            tile_position=(0, 0),
                        )
                        nc.tensor.matmul(
                            pv0[64:128, :], lhsT=v_sb[:, t - 1, hB, :],
                            rhs=pB[:, 0:512], start=st, stop=sp_,
                            tile_position=(0, 64),
                        )
                        nc.tensor.matmul(
                            pv1[0:64, :], lhsT=v_sb[:, t - 1, hA, :],
                            rhs=pA[:, 512:1024], start=st, stop=sp_,
                            tile_position=(0, 0),
                        )
                        nc.tensor.matmul(
                            pv1[64:128, :], lhsT=v_sb[:, t - 1, hB, :],
                            rhs=pB[:, 512:1024], start=st, stop=sp_,
                            tile_position=(0, 64),
                        )
                        nc.tensor.matmul(
                            pden[0:1, :], lhsT=ones_sb[:, :],
                            rhs=pA[:, 0:512], start=st, stop=sp_,
                            tile_position=(0, 0),
                        )
                        nc.tensor.matmul(
                            pden[32:33, :], lhsT=ones_sb[:, :],
                            rhs=pB[:, 0:512], start=st, stop=sp_,
                            tile_position=(0, 32),
                        )
                        nc.tensor.matmul(
                            pden[64:65, :], lhsT=ones_sb[:, :],
                            rhs=pA[:, 512:1024], start=st, stop=sp_,
                            tile_position=(0, 64),
                        )
                        nc.tensor.matmul(
                            pden[96:97, :], lhsT=ones_sb[:, :],
                            rhs=pB[:, 512:1024], start=st, stop=sp_,
                            tile_position=(0, 96),
                        )
                    if t < 16:
                        prev = (ptA, ptB)
                # evacuate PSUM: denominators first (frees pden for the
                # boundary projections), then the two O banks.
                dmae = nc.scalar if hp == 3 else nc.sync
                den_sb = norm.tile([128, 512], mybir.dt.float32, tag="den")
                nc.scalar.copy(den_sb[:, :], pden[:, :])
                oa = norm.tile([128, NQ], mybir.dt.float32, tag="oa")
                nc.scalar.copy(oa[:, 0:512], pv0[:, :])
                nc.vector.tensor_copy(oa[:, 512:NQ], pv1[:, :])
                # normalization: spread each head's 1024 sums across the
                # partitions for a wide reciprocal, then broadcast via a
                # DRAM round-trip (step-0 access pattern).  A's recip lands
                # on partitions 0:64 of bc, B's on 64:128, so one multiply
                # normalizes the whole pair in place.
                sp = norm.tile([128, 16], mybir.dt.float32, tag="sp", bufs=4)
                dmae.dma_start(sp[0:64, 0:8], den_sb[0:1, :])
                dmae.dma_start(sp[64:128, 0:8], den_sb[64:65, :])
                dmae.dma_start(sp[0:64, 8:16], den_sb[32:33, :])
                dmae.dma_start(sp[64:128, 8:16], den_sb[96:97, :])
                rsp = norm.tile([128, 16], mybir.dt.float32, tag="rsp", bufs=4)
                nc.vector.reciprocal(rsp[:, :], sp[:, :])
                sdA = dram.tile([1, NQ], mybir.dt.float32, tag="sdA", bufs=4)
                sdB = dram.tile([1, NQ], mybir.dt.float32, tag="sdB", bufs=4)
                dmae.dma_start(sdA[:, :], rsp[:, 0:8])
                dmae.dma_start(sdB[:, :], rsp[:, 8:16])
                bc = norm.tile([128, NQ], mybir.dt.float32, tag="bc", bufs=4)
                bcA = bass.AP(tensor=sdA.tensor, offset=sdA.offset,
                              ap=[[0, 64], [1, NQ]])
                bcB = bass.AP(tensor=sdB.tensor, offset=sdB.offset,
                              ap=[[0, 64], [1, NQ]])
                dmae.dma_start(bc[0:64, :], bcA)
                dmae.dma_start(bc[64:128, :], bcB)
                nc.vector.tensor_mul(ot_sb[:, hp, :], oa[:, :], bc[:, :])
                # next pair's projections ride the pair boundary: the po ring
                # slots are freeing up (oa copies done) and the PE has a gap
                # until the next pair's first PV
                if hp < 2:
                    q_proj(hp + 2)
                    k_proj(hp + 2, 0)
                    k_proj(hp + 2, 1)
                    swap_kq(hp + 2)
            # ---- tail: full output projection Y^T = Wout^T @ O^T.
            # Pairs 0..2 are accumulated for ALL e-tiles as soon as the last
            # pair's score/exp traffic frees the PSUM slots -- this runs
            # UNDER pair 3's normalization DMA round-trips.  Only the pair-3
            # contraction, bias and store wait for the final norm.
            tail_ps = []
            for m in range(4):
                if m == 0:
                    ps = psum_s.tile([128, NQ], mybir.dt.float32, tag="ssA",
                                     name=f"psyT{m}")
                    chunks = [ps[:, 0:512], ps[:, 512:NQ]]
                elif m == 1:
                    ps = psum_s.tile([128, NQ], mybir.dt.float32, tag="ssB",
                                     name=f"psyT{m}")
                    chunks = [ps[:, 0:512], ps[:, 512:NQ]]
                else:
                    c0 = psum_o.tile([128, 512], mybir.dt.float32, tag="po",
                                     name=f"psyT{m}c0")
                    c1 = psum_o.tile([128, 512], mybir.dt.float32, tag="po",
                                     name=f"psyT{m}c1")
                    chunks = [c0[:, :], c1[:, :]]
                for c in range(2):
                    cs = ts(c, 512)
                    for hp in range(3):
                        nc.tensor.matmul(
                            chunks[c],
                            lhsT=wout_sb[:, hp, ts(m, 128)],
                            rhs=ot_sb[:, hp, cs],
                            start=(hp == 0), stop=False,
                        )
                tail_ps.append(chunks)
            for m in range(4):
                chunks = tail_ps[m]
                ys = ysb.tile([128, NQ], mybir.dt.float32, tag="ys", bufs=4)
                for c in range(2):
                    cs = ts(c, 512)
                    nc.tensor.matmul(
                        chunks[c],
                        lhsT=wout_sb[:, 3, ts(m, 128)],
                        rhs=ot_sb[:, 3, cs],
                        start=False, stop=True,
                    )
                    nc.vector.tensor_scalar_add(ys[:, cs], chunks[c],
                                                bout_sb[:, m:m + 1])
                    nc.sync.dma_start(yt[ts(m, 128), cs], ys[:, cs])

    nc.compile()
    return nc


def _get_nc():
    if "nc" not in _CACHE:
        _CACHE["nc"] = _build()
    return _CACHE["nc"]


def kernel(x, w_qkv, w_out, b_out):
    global LAST_EXEC_TIME_NS
    from concourse.bass_utils import run_bass_kernel_spmd

    x = np.asarray(x, dtype=np.float32)
    w_qkv = np.asarray(w_qkv, dtype=np.float32)
    w_out = np.asarray(w_out, dtype=np.float32)
    b_out = np.asarray(b_out, dtype=np.float32)

    wqkv_b = w_qkv.astype(BF16)
    wout_b = w_out.astype(BF16)
    bout_t = np.ascontiguousarray(b_out.reshape(4, 128).T).astype(np.float32)

    in_maps = []
    for c in range(NCORES):
        b, qh = c // 2, c % 2
        q0 = qh * NQ
        xb = x[b]
        perm = np.concatenate([
            np.arange(q0, q0 + NQ),
            np.arange(0, q0),
            np.arange(q0 + NQ, N),
        ])
        xt = np.ascontiguousarray(xb[perm].T).astype(BF16)
        in_maps.append({
            "xt": xt,
            "wqkv": wqkv_b,
            "wout": wout_b,
            "bout": bout_t,
        })

    nc = _get_nc()
    trace = bool(int(os.environ.get("ATTN_TRACE", "0")))
    res = run_bass_kernel_spmd(nc, in_maps, core_ids=list(range(NCORES)),
                               trace=trace)
    LAST_EXEC_TIME_NS = res.exec_time_ns

    out = np.empty((B, N, DIM), np.float32)
    for c in range(NCORES):
        b, qh = c // 2, c % 2
        out[b, qh * NQ:(qh + 1) * NQ, :] = res.results[c]["out"].T
    return out



# revision 11
# speedup vs baseline: 1.0324x; 1.0324x over previous
"""Multi-head attention (B=4, N=2048, DIM=512, H=8) on 8 TRN2 NeuronCores.

Sharding: core c handles (batch = c//2, query-half = c%2) -> 1024 queries of
one batch, all heads. Zero collectives: K/V are recomputed per core pair
(keys are permuted so each core's queries come first; softmax is
permutation-invariant over keys).

Device layout ("transposed flash"):
  - everything dim-major: X^T, Q^T, K^T in SBUF with the contraction dim on
    partitions; V in natural [n, d] layout with a fused ones-column so the
    PV matmul also produces the softmax denominators (row 64 of the PSUM
    accumulator).
  - scores S^T = K_h^T-stationary @ Q_h^T-moving -> [nk_tile, nq] PSUM.
    The two heads of a pair live on disjoint PE row groups (d on partitions
    0:64 vs 64:128); their score matmuls are emitted ADJACENTLY
    (A-c0, B-c0, A-c1, B-c1) so the PE runs them concurrently as 64x128
    row tiles (2x throughput -- verified on HW: 111 ns vs 217 ns per MM).
  - exp on ScalarE (scale fused) -> P^T bf16 in SBUF. A tunable subset of
    head-B tiles runs instead on VectorE via a custom 8-slice DVE op
    computing exp(x) ~= ((1 + u + u^2/2)^8, u = x/8 (rel err < 2e-3 over
    the logit range; softmax denominator cancels most of it), offloading
    the ScalarE bottleneck.
  - O^T_aug += V_aug^T-stationary @ P^T-moving accumulated over nk tiles.
  - normalization: reciprocal of the sums row, broadcast across partitions
    via a DRAM round-trip DMA (step-0 access pattern), multiply on VectorE.
  - output projection Wout-stationary gives Y^T [512, 1024]; host transposes.
"""

import os

import numpy as np
import ml_dtypes

B, N, DIM = 4, 2048, 512
H, D = 8, 64
NQ = 1024            # queries per core
NCORES = 8
SCALE = DIM ** -0.5  # reference scales by full dim, not head dim

BF16 = ml_dtypes.bfloat16

_CACHE = {}

LAST_EXEC_TIME_NS = None

# head-B's exp runs on the DVE for ALL k-tiles: symmetric engine use makes
# both ss PSUM slots free at the same time, so the next tile's A/B score
# matmuls become ready together and the PE pairs them (64x128 row tiles)
EXP8_C1 = 0.51        # tuned quadratic coefficient


def _register_exp8():
    """Register the EXP8_APPROX_ANT custom DVE op (idempotent).

    out = ((C1*u + 1) * u + 1)^8 with u = Src0 * C0; C0 folds the softmax
    scale / 8.  Exactly 8 ALU slices -- fits the v3 (TRN2) DVE pipeline.
    """
    import concourse.dve_ops as dve_ops
    from concourse.dve_spec import C0, C1, C2, Spec, lower, sq
    from concourse.dve_ops import DveOp
    from concourse.dve_uop import DveOpSpec

    if "EXP8_APPROX_ANT" in dve_ops._SUB_OPCODE_FOR_NAME:
        return next(op for op in dve_ops.OPS if op.name == "EXP8_APPROX_ANT")

    from concourse.dve_spec import Src0

    u = Src0 * C0
    body = sq(sq(sq((u * C1 + C2) * u + C2)))

    def _ref(in0, in1, s0, s1, imm2):
        uu = in0.astype(np.float32) * s0
        p = (uu * s1 + imm2) * uu + imm2
        return ((p * p) ** 2) ** 2

    spec = Spec(body=body, reference=_ref)

    row = max(dve_ops._SUB_OPCODE_FOR_NAME.values()) + 1
    assert row < 0x20

    # pin the sha by lowering once per version
    shas = {}
    for ver in ("v3", "v4"):
        try:
            uops = lower(spec, ver=ver)
            shas[ver] = DveOpSpec(
                name="EXP8_APPROX_ANT", opcode=row, uops=uops, rd1_en=False
            ).sha(ver)
        except Exception:
            pass

    op = DveOp("EXP8_APPROX_ANT", spec, subdim=False, uops_sha=shas)
    dve_ops.OPS.append(op)
    dve_ops.CUSTOM_DVE_SPECS[op.name] = spec
    dve_ops._SUB_OPCODE_FOR_NAME[op.name] = row
    return op


def _build():
    import concourse.bass as bass
    import concourse.mybir as mybir
    import concourse.tile as tile
    from concourse import bacc

    f32 = mybir.dt.float32
    bf16 = mybir.dt.bfloat16
    Exp = mybir.ActivationFunctionType.Exp
    ts = bass.ts

    exp8 = _register_exp8()
    exp8_s0 = SCALE / 8.0
    use_dve_exp = not bool(int(os.environ.get("ATTN_NO_DVE_EXP", "0")))

    nc = bacc.Bacc("TRN2", target_bir_lowering=False, debug=False,
                   num_devices=NCORES)

    xt = nc.dram_tensor("xt", [DIM, N], bf16, kind="ExternalInput")
    wqkv = nc.dram_tensor("wqkv", [DIM, 3 * DIM], bf16, kind="ExternalInput")
    wout = nc.dram_tensor("wout", [DIM, DIM], bf16, kind="ExternalInput")
    bout = nc.dram_tensor("bout", [128, 4], f32, kind="ExternalInput")
    yt = nc.dram_tensor("out", [DIM, NQ], f32, kind="ExternalOutput")

    with tile.TileContext(nc) as tc:
        with (
            tc.tile_pool(name="persist", bufs=1) as persist,
            tc.tile_pool(name="ptiles", bufs=8) as ptiles,
            tc.tile_pool(name="norm", bufs=2) as norm,
            tc.tile_pool(name="ysb", bufs=2) as ysb,
            tc.tile_pool(name="psum_s", bufs=1, space="PSUM") as psum_s,
            tc.tile_pool(name="psum_o", bufs=4, space="PSUM") as psum_o,
            tc.tile_pool(name="dram", bufs=2, space="DRAM") as dram,
        ):
            # ---- load inputs (fine-grained DMAs spread across queues) ----
            # Issue order = queue assignment order: land the chunks the first
            # projection (K tile 0, then Q, m=0) needs before everything else.
            xt_sb = persist.tile([128, 4, N], bf16)
            w_sb = persist.tile([128, 4, 3 * DIM], bf16)
            # critical path first, on distinct queues: the K m=0 / Q m=0
            # weight slivers and x^T (full rows; descriptor count is per
            # partition, so chunking columns does not speed a queue up)
            def ld(i, dst, srcs):
                eng = nc.sync if i % 2 == 0 else nc.scalar
                eng.dma_start(dst, srcs)
            for kt in range(4):
                ld(kt, w_sb[:, kt, 512:640], wqkv[ts(kt, 128), 512:640])
            for kt in range(4):
                ld(kt, xt_sb[:, kt, 0:1024], xt[ts(kt, 128), 0:1024])
            for kt in range(4):
                ld(kt + 1, w_sb[:, kt, 0:128], wqkv[ts(kt, 128), 0:128])
            for kt in range(4):
                ld(kt + 1, xt_sb[:, kt, 1024:2048], xt[ts(kt, 128), 1024:2048])
            for kt in range(4):
                ld(kt, w_sb[:, kt, ts(2, 512)], wqkv[ts(kt, 128), ts(2, 512)])
            for kt in range(4):
                ld(kt + 1, w_sb[:, kt, 640:1024], wqkv[ts(kt, 128), 640:1024])
            for kt in range(4):
                ld(kt, w_sb[:, kt, 128:512], wqkv[ts(kt, 128), 128:512])
            wout_sb = persist.tile([128, 4, DIM], bf16)
            for kt in range(4):
                ld(kt + 1, wout_sb[:, kt, :], wout[ts(kt, 128), :])
            bout_sb = persist.tile([128, 4], f32)
            nc.sync.dma_start(bout_sb[:], bout[:, :])

            qt_sb = persist.tile([128, 4, NQ], bf16)
            kt_sb = persist.tile([128, 4, N], bf16)
            # partition-swapped copies (head A on rows 64:128, B on rows
            # 0:64): chunk c0 of a head's score matmul runs on one PE row
            # half and chunk c1 on the other, so the two chunks pair on the
            # PE regardless of scheduler phase.
            qt_sw = persist.tile([128, 4, NQ], bf16)
            kt_sw = persist.tile([128, 4, N], bf16)

            def swap_kq(m):
                # SBUF->SBUF partition-shift DMAs, chunked for early tiles
                for c0 in range(0, N, 512):
                    nc.sync.dma_start(kt_sw[64:128, m, c0:c0 + 512],
                                      kt_sb[0:64, m, c0:c0 + 512])
                    nc.sync.dma_start(kt_sw[0:64, m, c0:c0 + 512],
                                      kt_sb[64:128, m, c0:c0 + 512])
                for c0 in range(0, NQ, 512):
                    nc.sync.dma_start(qt_sw[64:128, m, c0:c0 + 512],
                                      qt_sb[0:64, m, c0:c0 + 512])
                    nc.sync.dma_start(qt_sw[0:64, m, c0:c0 + 512],
                                      qt_sb[64:128, m, c0:c0 + 512])

            def q_proj(m, split_copy=False):
                # Q^T tile m: stationary = Wq tile, moving = X^T.
                # Two single-bank chunk tiles so the po ring stays 1-bank
                # granular (pv0/pv1/pden share the same 4-slot ring).
                ps0 = psum_o.tile([128, 512], mybir.dt.float32, tag="po",
                                  name=f"psq{m}c0")
                ps1 = psum_o.tile([128, 512], mybir.dt.float32, tag="po",
                                  name=f"psq{m}c1")
                pss = (ps0, ps1)
                for kt in range(4):
                    for c in range(2):
                        nc.tensor.matmul(
                            pss[c][:, :],
                            lhsT=w_sb[:, kt, ts(m, 128)],
                            rhs=xt_sb[:, kt, ts(c, 512)],
                            start=(kt == 0), stop=(kt == 3),
                        )
                if split_copy:
                    # first chunk unblocks the first score matmuls sooner
                    nc.vector.tensor_copy(qt_sb[:, m, 0:512], ps0[:, :])
                    nc.vector.tensor_copy(qt_sb[:, m, 512:NQ], ps1[:, :])
                else:
                    nc.scalar.copy(qt_sb[:, m, 0:512], ps0[:, :])
                    nc.scalar.copy(qt_sb[:, m, 512:NQ], ps1[:, :])

            def k_proj(m, cc, split_copy=False):
                ps0 = psum_o.tile([128, 512], mybir.dt.float32, tag="po",
                                  name=f"psk{m}_{cc}c0")
                ps1 = psum_o.tile([128, 512], mybir.dt.float32, tag="po",
                                  name=f"psk{m}_{cc}c1")
                pss = (ps0, ps1)
                for kt in range(4):
                    for c in range(2):
                        nc.tensor.matmul(
                            pss[c][:, :],
                            lhsT=w_sb[:, kt, 512 + m * 128:512 + (m + 1) * 128],
                            rhs=xt_sb[:, kt, cc * 1024 + c * 512:cc * 1024 + (c + 1) * 512],
                            start=(kt == 0), stop=(kt == 3),
                        )
                base = cc * 1024
                if split_copy:
                    nc.scalar.copy(kt_sb[:, m, base:base + 256],
                                   ps0[:, 0:256])
                    nc.scalar.copy(kt_sb[:, m, base + 256:base + 512],
                                   ps0[:, 256:512])
                    nc.scalar.copy(kt_sb[:, m, base + 512:base + 1024],
                                   ps1[:, :])
                else:
                    nc.scalar.copy(kt_sb[:, m, base:base + 512], ps0[:, :])
                    nc.scalar.copy(kt_sb[:, m, base + 512:base + 1024],
                                   ps1[:, :])

            k_proj(0, 0, split_copy=True)
            q_proj(0, split_copy=True)
            k_proj(0, 1)
            swap_kq(0)
            q_proj(1)
            k_proj(1, 0)
            k_proj(1, 1)
            swap_kq(1)

            # V natural [2048, 512] -> v_sb [128, nk_tile, head, 64]; the
            # softmax denominators come from separate ones-matmuls (so the
            # per-head PV stationary stays 64-wide and two heads pair on
            # disjoint PE column groups).
            v_sb = persist.tile([128, 16, H, D], bf16)
            ones_sb = persist.tile([128, 1], bf16)
            nc.vector.memset(ones_sb[:, :], 1.0)

            def v_proj():
                for t in range(16):
                    ps = psum_o.tile([128, 512], mybir.dt.float32, tag="po",
                                     name=f"psv{t}")
                    for kt in range(4):
                        nc.tensor.matmul(
                            ps[:, :],
                            lhsT=xt_sb[:, kt, ts(t, 128)],
                            rhs=w_sb[:, kt, 1024:1536],
                            start=(kt == 0), stop=(kt == 3),
                        )
                    if t % 2 == 0:
                        nc.scalar.copy(
                            v_sb[:, t, :, :],
                            ps[:, :].rearrange("p (h d) -> p h d", h=H),
                        )
                    else:
                        nc.vector.tensor_copy(
                            v_sb[:, t, :, :],
                            ps[:, :].rearrange("p (h d) -> p h d", h=H),
                        )

            # ---- attention, one head PAIR at a time ----
            # Head A lives on partitions 0:64, head B on 64:128 of K^T/Q^T
            # tile hp.  Both heads' scores for one k-tile land in ONE
            # [128, 2048] PSUM tile (A cols 0:1024, B cols 1024:2048): the
            # 4 score matmuls become ready atomically, so the scheduler
            # issues them back-to-back and the PE row-tiles A/B pairs
            # concurrently.  exp(A) on ScalarE reads banks 0-1 while
            # exp8(B) on VectorE reads banks 2-3 in parallel.
            ot_sb = persist.tile([128, 4, NQ], bf16)

            for hp in range(4):
                hA, hB = 2 * hp, 2 * hp + 1
                # po accumulators allocated lazily (for pair 0 they must come
                # AFTER the V-projection's psum_o allocations)
                pv0 = pv1 = pden = None
                # Software-pipelined: scores/exp for tile t are emitted one
                # iteration AHEAD of the PV matmuls for tile t-1, so freshly
                # unblocked score matmuls sit at the head of the PE FIFO
                # instead of behind the PV work (keeps ScalarE saturated).
                prev = None
                for t in range(17):
                    if t < 16:
                        # single-bank score chunks: exp consumes and releases
                        # each bank separately, so the next tile's score
                        # matmul for a chunk can start while the OTHER chunk's
                        # exp is still running -- the t-loop cadence becomes
                        # max(engine exp rate, PE work) instead of
                        # exp + scores serialized on a monolithic tile.
                        sA0 = psum_s.tile([128, 512], mybir.dt.float32,
                                          tag="sA0", name="sA0")
                        sA1 = psum_s.tile([128, 512], mybir.dt.float32,
                                          tag="sA1", name="sA1")
                        sB0 = psum_s.tile([128, 512], mybir.dt.float32,
                                          tag="sB0", name="sB0")
                        sB1 = psum_s.tile([128, 512], mybir.dt.float32,
                                          tag="sB1", name="sB1")
                        # sacrificial N=1 matmuls: they carry the PSUM-slot
                        # release waits (and the 128->64 tile-mode switch), so
                        # the real score matmuls below issue wait-free and the
                        # PE runs the A/B row-tile pairs concurrently
                        nc.tensor.matmul(
                            sA0[:, 0:1],
                            lhsT=kt_sb[0:64, hp, ts(t, 128)],
                            rhs=qt_sb[0:64, hp, 0:1],
                            start=True, stop=True,
                        )
                        nc.tensor.matmul(
                            sB0[:, 0:1],
                            lhsT=kt_sb[64:128, hp, ts(t, 128)],
                            rhs=qt_sb[64:128, hp, 0:1],
                            start=True, stop=True,
                        )
                        # c0 on one PE row half, c1 on the other (swapped
                        # copies) -> the two chunks run concurrently
                        nc.tensor.matmul(
                            sA0[:, :],
                            lhsT=kt_sb[0:64, hp, ts(t, 128)],
                            rhs=qt_sb[0:64, hp, 0:512],
                            start=True, stop=True,
                        )
                        nc.tensor.matmul(
                            sA1[:, :],
                            lhsT=kt_sw[64:128, hp, ts(t, 128)],
                            rhs=qt_sw[64:128, hp, 512:1024],
                            start=True, stop=True,
                        )
                        nc.tensor.matmul(
                            sB0[:, :],
                            lhsT=kt_sb[64:128, hp, ts(t, 128)],
                            rhs=qt_sb[64:128, hp, 0:512],
                            start=True, stop=True,
                        )
                        nc.tensor.matmul(
                            sB1[:, :],
                            lhsT=kt_sw[0:64, hp, ts(t, 128)],
                            rhs=qt_sw[0:64, hp, 512:1024],
                            start=True, stop=True,
                        )
                        ptA = ptiles.tile([128, NQ], bf16, tag="pt")
                        ptB = ptiles.tile([128, NQ], bf16, tag="pt")
                        nc.scalar.activation(ptA[:, 0:512], sA0[:, :], Exp,
                                             scale=SCALE)
                        if use_dve_exp:
                            nc.vector._custom_dve(
                                exp8, out=ptB[:, 0:512], in0=sB0[:, :],
                                s0=exp8_s0, s1=EXP8_C1, imm2=1.0,
                            )
                        else:
                            nc.scalar.activation(ptB[:, 0:512], sB0[:, :],
                                                 Exp, scale=SCALE)
                        nc.scalar.activation(ptA[:, 512:NQ], sA1[:, :], Exp,
                                             scale=SCALE)
                        if use_dve_exp:
                            nc.vector._custom_dve(
                                exp8, out=ptB[:, 512:NQ], in0=sB1[:, :],
                                s0=exp8_s0, s1=EXP8_C1, imm2=1.0,
                            )
                        else:
                            nc.scalar.activation(ptB[:, 512:NQ], sB1[:, :],
                                                 Exp, scale=SCALE)
                    # V-projection rides here for pair 0: the first score
                    # group + exp are already emitted, so ScalarE ramps up
                    # while the PE grinds through the V matmuls.
                    if hp == 0 and t == 0:
                        v_proj()
                    if t >= 1:
                        if pv0 is None:
                            # pv0 = c0 bank (A rows 0:64, B rows 64:128),
                            # pv1 = c1 bank, pden = denominator bank with
                            # rows {0: A-c0, 32: B-c0, 64: A-c1, 96: B-c1}
                            pv0 = psum_o.tile([128, 512], mybir.dt.float32,
                                              tag="po", name="pv0")
                            pv1 = psum_o.tile([128, 512], mybir.dt.float32,
                                              tag="po", name="pv1")
                            pden = psum_o.tile([128, 512], mybir.dt.float32,
                                               tag="po", name="pden")
                        pA, pB = prev
                        st, sp_ = (t == 1), (t == 16)
                        # column-group-balanced schedule: each 32-col PE
                        # group sees exactly 3x512 moving columns per k-tile
                        # (two 64-row V matmuls + one 1-row ones matmul),
                        # emitted in col-disjoint pairs so they overlap.
                        nc.tensor.matmul(
                            pv0[0:64, :], lhsT=v_sb[:, t - 1, hA, :],
                            rhs=pA[:, 0:512], start=st, stop=sp_,
                            tile_position=(0, 0),
                        )
                        nc.tensor.matmul(
                            pv0[64:128, :], lhsT=v_sb[:, t - 1, hB, :],
                            rhs=pB[:, 0:512], start=st, stop=sp_,
                            tile_position=(0, 64),
                        )
                        nc.tensor.matmul(
                            pv1[0:64, :], lhsT=v_sb[:, t - 1, hA, :],
                            rhs=pA[:, 512:1024], start=st, stop=sp_,
                            tile_position=(0, 0),
                        )
                        nc.tensor.matmul(
                            pv1[64:128, :], lhsT=v_sb[:, t - 1, hB, :],
                            rhs=pB[:, 512:1024], start=st, stop=sp_,
                            tile_position=(0, 64),
                        )
                        nc.tensor.matmul(
                            pden[0:1, :], lhsT=ones_sb[:, :],
                            rhs=pA[:, 0:512], start=st, stop=sp_,
                            tile_position=(0, 0),
                        )
                        nc.tensor.matmul(
                            pden[32:33, :], lhsT=ones_sb[:, :],
                            rhs=pB[:, 0:512], start=st, stop=sp_,
                            tile_position=(0, 32),
                        )
                        nc.tensor.matmul(
                            pden[64:65, :], lhsT=ones_sb[:, :],
                            rhs=pA[:, 512:1024], start=st, stop=sp_,
                            tile_position=(0, 64),
                        )
                        nc.tensor.matmul(
                            pden[96:97, :], lhsT=ones_sb[:, :],
                            rhs=pB[:, 512:1024], start=st, stop=sp_,
                            tile_position=(0, 96),
                        )
                    if t < 16:
                        prev = (ptA, ptB)
                # evacuate PSUM: denominators first (frees pden for the
                # boundary projections), then the two O banks.
                dm0 = nc.scalar if hp == 3 else nc.sync
                dm1 = nc.gpsimd
                den_sb = norm.tile([128, 512], mybir.dt.float32, tag="den")
                nc.scalar.copy(den_sb[:, :], pden[:, :])
                oa = norm.tile([128, NQ], mybir.dt.float32, tag="oa")
                nc.scalar.copy(oa[:, 0:512], pv0[:, :])
                nc.vector.tensor_copy(oa[:, 512:NQ], pv1[:, :])
                # normalization: spread each head's 1024 sums across the
                # partitions for a wide reciprocal, then broadcast via a
                # DRAM round-trip (step-0 access pattern).  A's recip lands
                # on partitions 0:64 of bc, B's on 64:128, so one multiply
                # normalizes the whole pair in place.  Two DMA queues split
                # the latency chain.
                sp = norm.tile([128, 16], mybir.dt.float32, tag="sp", bufs=4)
                dm0.dma_start(sp[0:64, 0:8], den_sb[0:1, :])
                dm0.dma_start(sp[64:128, 0:8], den_sb[64:65, :])
                dm1.dma_start(sp[0:64, 8:16], den_sb[32:33, :])
                dm1.dma_start(sp[64:128, 8:16], den_sb[96:97, :])
                rsp = norm.tile([128, 16], mybir.dt.float32, tag="rsp", bufs=4)
                nc.vector.reciprocal(rsp[:, :], sp[:, :])
                sdA = dram.tile([1, NQ], mybir.dt.float32, tag="sdA", bufs=4)
                sdB = dram.tile([1, NQ], mybir.dt.float32, tag="sdB", bufs=4)
                dm0.dma_start(sdA[:, :], rsp[:, 0:8])
                dm1.dma_start(sdB[:, :], rsp[:, 8:16])
                bc = norm.tile([128, NQ], mybir.dt.float32, tag="bc", bufs=4)
                bcA = bass.AP(tensor=sdA.tensor, offset=sdA.offset,
                              ap=[[0, 64], [1, NQ]])
                bcB = bass.AP(tensor=sdB.tensor, offset=sdB.offset,
                              ap=[[0, 64], [1, NQ]])
                dm0.dma_start(bc[0:64, :], bcA)
                dm1.dma_start(bc[64:128, :], bcB)
                nc.vector.tensor_mul(ot_sb[:, hp, :], oa[:, :], bc[:, :])
                # next pair's projections ride the pair boundary: the po ring
                # slots are freeing up (oa copies done) and the PE has a gap
                # until the next pair's first PV
                if hp < 2:
                    q_proj(hp + 2)
                    k_proj(hp + 2, 0)
                    k_proj(hp + 2, 1)
                    swap_kq(hp + 2)
            # ---- tail: full output projection Y^T = Wout^T @ O^T.
            # Pairs 0..2 are accumulated for ALL e-tiles as soon as the last
            # pair's score/exp traffic frees the PSUM slots -- this runs
            # UNDER pair 3's normalization DMA round-trips.  Only the pair-3
            # contraction, bias and store wait for the final norm.
            tail_ps = []
            for m in range(4):
                if m == 0:
                    c0 = psum_s.tile([128, 512], mybir.dt.float32, tag="sA0",
                                     name=f"psyT{m}c0")
                    c1 = psum_s.tile([128, 512], mybir.dt.float32, tag="sA1",
                                     name=f"psyT{m}c1")
                    chunks = [c0[:, :], c1[:, :]]
                elif m == 1:
                    c0 = psum_s.tile([128, 512], mybir.dt.float32, tag="sB0",
                                     name=f"psyT{m}c0")
                    c1 = psum_s.tile([128, 512], mybir.dt.float32, tag="sB1",
                                     name=f"psyT{m}c1")
                    chunks = [c0[:, :], c1[:, :]]
                else:
                    c0 = psum_o.tile([128, 512], mybir.dt.float32, tag="po",
                                     name=f"psyT{m}c0")
                    c1 = psum_o.tile([128, 512], mybir.dt.float32, tag="po",
                                     name=f"psyT{m}c1")
                    chunks = [c0[:, :], c1[:, :]]
                for c in range(2):
                    cs = ts(c, 512)
                    for hp in range(3):
                        nc.tensor.matmul(
                            chunks[c],
                            lhsT=wout_sb[:, hp, ts(m, 128)],
                            rhs=ot_sb[:, hp, cs],
                            start=(hp == 0), stop=False,
                        )
                tail_ps.append(chunks)
            for m in range(4):
                chunks = tail_ps[m]
                ys = ysb.tile([128, NQ], mybir.dt.float32, tag="ys", bufs=4)
                for c in range(2):
                    cs = ts(c, 512)
                    nc.tensor.matmul(
                        chunks[c],
                        lhsT=wout_sb[:, 3, ts(m, 128)],
                        rhs=ot_sb[:, 3, cs],
                        start=False, stop=True,
                    )
                    nc.vector.tensor_scalar_add(ys[:, cs], chunks[c],
                                                bout_sb[:, m:m + 1])
                    nc.sync.dma_start(yt[ts(m, 128), cs], ys[:, cs])

    nc.compile()
    return nc


def _get_nc():
    if "nc" not in _CACHE:
        _CACHE["nc"] = _build()
    return _CACHE["nc"]


def kernel(x, w_qkv, w_out, b_out):
    global LAST_EXEC_TIME_NS
    from concourse.bass_utils import run_bass_kernel_spmd

    x = np.asarray(x, dtype=np.float32)
    w_qkv = np.asarray(w_qkv, dtype=np.float32)
    w_out = np.asarray(w_out, dtype=np.float32)
    b_out = np.asarray(b_out, dtype=np.float32)

    wqkv_b = w_qkv.astype(BF16)
    wout_b = w_out.astype(BF16)
    bout_t = np.ascontiguousarray(b_out.reshape(4, 128).T).astype(np.float32)

    in_maps = []
    for c in range(NCORES):
        b, qh = c // 2, c % 2
        q0 = qh * NQ
        xb = x[b]
        perm = np.concatenate([
            np.arange(q0, q0 + NQ),
            np.arange(0, q0),
            np.arange(q0 + NQ, N),
        ])
        xt = np.ascontiguousarray(xb[perm].T).astype(BF16)
        in_maps.append({
            "xt": xt,
            "wqkv": wqkv_b,
            "wout": wout_b,
            "bout": bout_t,
        })

    nc = _get_nc()
    trace = bool(int(os.environ.get("ATTN_TRACE", "0")))
    res = run_bass_kernel_spmd(nc, in_maps, core_ids=list(range(NCORES)),
                               trace=trace)
    LAST_EXEC_TIME_NS = res.exec_time_ns

    out = np.empty((B, N, DIM), np.float32)
    for c in range(NCORES):
        b, qh = c // 2, c % 2
        out[b, qh * NQ:(qh + 1) * NQ, :] = res.results[c]["out"].T
    return out

